# revision 37
# baseline (speedup 1.0000x reference)
"""Trainium2 Bass kernel for nn_ModelNew_3556232921835 (dense_mlp).

Reference computation:
    d = x @ W^T - subtract                      [M, N]
    c = mean(d, axis=1) + log(N)                [M, 1]
    g = gelu_fast_tanh(c)                       [M, 1]   (t/(|t|+1) surrogate)
    out = g + x                                 [M, N]

Key algebraic identity: the full GEMM is never needed.
    mean_n(x @ W^T - s) = (x . colsum(W)) / N - mean(s)
so the kernel computes v = colsum(W) (a [K] vector) once, then a per-row
dot product x[m,:] . v, the scalar gelu, and a broadcast add of x.

The kernel is DMA-bound (read x, write out). All streaming I/O is fp16:
out = g + x with g ~= 8.2 and |x| < 6 gives |out| in [2.7, 13.8]; fp16
rounding of both x and out contributes < 1e-2 abs err -> rel err ~5e-4
against the 2e-2 gate, while halving HBM traffic vs fp32.

Distribution (8 cores):
    x rows sharded M/8 = 2048 per core (data parallel).
    weight rows sharded N/8 = 512 per core -> partial colsum via PE matmul
    with a ones vector -> 8 KiB fp16 AllReduce add -> full v on every core.
    subtract replicated; each core reduces it locally.
No other cross-device communication.

Schedule (two-level software pipeline):
  * Rep level: rep r+1's preamble (weight quarters -> PE colsum ->
    AllReduce -> v_b broadcast, double-buffered) is emitted BEFORE rep
    r's main pass, so the ~tens-of-us collective latency hides under rep
    r's streaming work. The v_b broadcast waits on the collective, so it
    rides the otherwise-idle gpsimd ring -- on a streaming engine's ring
    that wait would park the engine's sequencer in front of the next
    main pass.
  * Chunk level: per 256-row chunk, head = two DVE fp16 tensor_tensor
    products (2x mode) + two ScalarE Copy+accum row-sums (1/(N*64) scale
    folds both the mean and the fp8 weight pre-scale); tail = short
    [128,2] gelu chain + two DVE tensor_scalar broadcast-adds (4x mode)
    + 2 MiB store. Tails trail heads by LOOKAHEAD chunks so the DVE
    sequencer never parks on a head-result semaphore.
  * Rings: x-loads stream on the SP HWDGE ring (emitted up front),
    out-stores on the ACT HWDGE ring, tiny scratch DMAs on the ACT ring
    in dependency order, v_b on gpsimd.

Weights ship as fp8 e4m3 scaled x64 on the host (entries ~N(0,1) land in
e4m3's normal range); x/out/v stream as fp16; all accumulation in fp32.
Measured ~92-123 us/rep steady-state (noise-dependent) vs the 391.6 us
session baseline; the measured no-collective DMA floor is ~90 us.
"""

import math
from contextlib import ExitStack

import numpy as np

M, K = 16384, 4096
N = 4096
NCORES = 8
M_LOC = M // NCORES      # 2048 x-rows per core
W_LOC = N // NCORES      # 512 weight rows per core
P = 128                  # SBUF partitions
ROWS_PER_CHUNK = 256     # 2 row-tiles per DMA chunk
NCHUNK = M_LOC // ROWS_PER_CHUNK   # 8 chunks per core
A_PER_CHUNK = ROWS_PER_CHUNK // P  # 2 row-tiles inside a chunk
MM_N = 512               # matmul free-dim chunk (one PSUM bank of f32)
ALPHA = float(np.sqrt(2.0 / np.pi))
C3 = 0.044715
# weights ship as fp8 e4m3, pre-scaled by 64 on the host so the ~N(0, 1/64^2)
# entries land in e4m3's normal range; the 1/64 is folded into the row-sum
# scale on ScalarE.
WSCALE = 64.0

_cached = {}


def _build_bass(reps=1, sim=False, coll="dram", vb_ring="gpsimd"):
    import concourse.bacc as bacc
    import concourse.tile as tile
    from concourse import mybir
    from concourse._compat import get_trn_type

    F32 = mybir.dt.float32
    F16 = mybir.dt.float16
    F8 = mybir.dt.float8e4
    # Bacc (not raw Bass): its compile() runs generate_event_semaphores,
    # which splits multi-wait instructions for the 1-wait-per-inst HW limit.
    nc = bacc.Bacc(
        get_trn_type() or "TRN2",
        target_bir_lowering=False,
        debug=False,
        num_devices=1 if sim else NCORES,
    )

    x = nc.dram_tensor("x", [M_LOC, K], F16, kind="ExternalInput")
    w = nc.dram_tensor("w", [W_LOC, K], F8, kind="ExternalInput")
    sub = nc.dram_tensor("sub", [1, N], F16, kind="ExternalInput")
    out = nc.dram_tensor("out", [M_LOC, K], F16, kind="ExternalOutput")
    vin = nc.dram_tensor("vin_scratch", [1, K], F16)
    if sim:
        vout = nc.dram_tensor("vout_scratch", [1, K], F16)
    else:
        vout = nc.dram_tensor("vout_scratch", [1, K], F16, addr_space="Shared")
    bsc = nc.dram_tensor("b_scratch", [1, 1], F32)

    with tile.TileContext(nc) as tc, ExitStack() as ctx:
      singles = ctx.enter_context(tc.tile_pool(name="singles", bufs=1))
      wpool = ctx.enter_context(tc.tile_pool(name="wpool", bufs=3))
      xpool = ctx.enter_context(tc.tile_pool(name="xpool", bufs=6))
      opool = ctx.enter_context(tc.tile_pool(name="opool", bufs=4))
      small = ctx.enter_context(tc.tile_pool(name="small", bufs=4))
      psum = ctx.enter_context(tc.tile_pool(name="psum", bufs=1, space="PSUM"))
      state = {}

      def preamble(_rep):
          ones = singles.tile([P, 1], F8)
          nc.vector.memset(ones, 1.0)

          # ---- SP HWDGE ring: weight quarters, then ALL x-loads (emitted up
          # front so HBM saturates while the preamble resolves). Each fp8
          # quarter's 8 column-chunk matmuls accumulate into the same PSUM
          # banks across the four quarters.
          JROWS = W_LOC // P  # 4 row-blocks of the weight shard
          pv = psum.tile([1, K], F32)

          def load_wq(j):
              wq = wpool.tile([P, K], F8, tag="wq", name=f"wq{j}")
              nc.sync.dma_start(
                  out=wq, in_=w[j * P:(j + 1) * P, :])
              return wq

          def wq_matmuls(j, wq):
              for c in range(K // MM_N):  # 8 column chunks
                  nc.tensor.matmul(
                      pv[0:1, c * MM_N:(c + 1) * MM_N],
                      lhsT=ones,
                      rhs=wq[:, c * MM_N:(c + 1) * MM_N],
                      start=(j == 0),
                      stop=(j == JROWS - 1),
                  )

          # all four weight quarters go first: they gate the long preamble
          # chain (PE colsum -> AllReduce -> v_b). With bufs=3 only wq3
          # briefly waits for wq0's matmuls to retire.
          wqs = {j: load_wq(j) for j in range(JROWS)}
          s_row = singles.tile([1, N], F16)
          nc.sync.dma_start(out=s_row, in_=sub[0:1, :])
          xts = []

          def load_x(i):
              r0 = i * ROWS_PER_CHUNK
              xt = xpool.tile([P, A_PER_CHUNK, K], F16, tag="xt",
                              name=f"xt{i}")
              # partition p holds rows 2p, 2p+1: each partition's 16 KiB
              # is contiguous in DRAM -> half the DMA descriptors of the
              # (a p) interleave
              nc.sync.dma_start(
                  out=xt,
                  in_=x[r0:r0 + ROWS_PER_CHUNK, :].rearrange(
                      "(p a) k -> p a k", a=A_PER_CHUNK),
              )
              xts.append(xt)

          for i in range(NCHUNK):
              load_x(i)

          # ---- partial colsum of the weight shard via PE: v_part = ones^T @ w
          for j in range(JROWS):
              wq_matmuls(j, wqs[j])
          # ---- bias b = log(N) - mean(subtract), replicated per partition.
          # All tiny scratch DMAs ride the low-latency ACT HWDGE ring, in
          # dependency order; the gpsimd ring carries only the collective.
          ssum = small.tile([1, 1], F32)
          nc.vector.tensor_reduce(
              out=ssum, in_=s_row, axis=mybir.AxisListType.X,
              op=mybir.AluOpType.add,
          )
          bb0 = small.tile([1, 1], F32)
          nc.vector.tensor_scalar(
              out=bb0, in0=ssum, scalar1=-1.0 / N, scalar2=math.log(N),
              op0=mybir.AluOpType.mult, op1=mybir.AluOpType.add,
          )
          nc.scalar.dma_start(out=bsc[0:1, :], in_=bb0)
          # bb rotates in the small pool: a bufs=1 slot would chain rep r+1's
          # bias DMA onto rep r's very last tail
          bb = small.tile([P, 1], F32, tag="bb")
          nc.scalar.dma_start(out=bb, in_=bsc[0:1, :].to_broadcast([P, 1]))

          vrow = singles.tile([1, K], F16)
          nc.scalar.copy(out=vrow, in_=pv)

          # ---- AllReduce the partial colsums across the 8 cores ----
          nc.scalar.dma_start(out=vin[0:1, :], in_=vrow)
          if sim or coll == "none":
              # timing experiment only: local copy instead of the collective
              nc.scalar.dma_start(out=vout[0:1, :], in_=vin[0:1, :])
          else:
              nc.gpsimd.collective_compute(
                  "AllReduce",
                  mybir.AluOpType.add,
                  replica_groups=[list(range(NCORES))],
                  ins=[vin[0:1, :]],
                  outs=[vout[0:1, :]],
              )
          # broadcast-read the reduced v into all 128 partitions.
          # This ride MUST stay on the gpsimd ring: it waits on the
          # collective, and on the ACT ring that wait would park the ACT
          # sequencer in front of the next main pass's row-sums.
          v_b = singles.tile([P, K], F16, bufs=2)
          vb_eng = nc.gpsimd if vb_ring == "gpsimd" else nc.scalar
          vb_eng.dma_start(out=v_b, in_=vout[0:1, :].to_broadcast([P, K]))

          state[_rep] = (xts, v_b, bb)

      def main_pass(_rep):
          xts, v_b, bb = state.pop(_rep)
          # ---- main pass over x row-chunks, software-pipelined ----
          # head(i): DVE products + ACT row-sums; tail(i): gelu + add +
          # store. Tails are emitted LOOKAHEAD chunks behind heads so the
          # DVE sequencer never parks on a head-result semaphore while
          # later heads could already run.
          LOOKAHEAD = 2
          live = {}

          def head(i):
              xt = xts[i]
              ot = opool.tile([P, A_PER_CHUNK, K], F16, tag="ot",
                              name=f"ot{i}")
              ysum = small.tile([P, A_PER_CHUNK], F32, tag="ysum",
                                name=f"ysum{i}")
              for a in range(A_PER_CHUNK):
                  # product on DVE at 2x (fp16 tensor_tensor); lands in ot
                  # as scratch -- the row-sum Copy and the final add both
                  # overwrite it in place
                  nc.vector.tensor_mul(ot[:, a, :], xt[:, a, :], v_b)
              for a in range(A_PER_CHUNK):
                  # row-sum on ScalarE: accum_out = sum(ot/N) per partition
                  nc.scalar.activation(
                      out=ot[:, a, :], in_=ot[:, a, :],
                      func=mybir.ActivationFunctionType.Copy,
                      bias=0.0, scale=1.0 / (N * WSCALE),
                      accum_out=ysum[:, a:a + 1],
                  )
              live[i] = (ot, ysum)

          def tail(i):
              xt = xts[i]
              ot, ysum = live.pop(i)
              y = small.tile([P, A_PER_CHUNK], F32)
              nc.vector.tensor_scalar_add(out=y, in0=ysum, scalar1=bb)
              # fast-tanh gelu on [P,2]: g = 0.5*y*(1 + t/(|t|+1)),
              # t = a*y*(1 + c3*y^2)
              y2 = small.tile([P, A_PER_CHUNK], F32)
              nc.vector.tensor_mul(y2, y, y)
              pp = small.tile([P, A_PER_CHUNK], F32)
              nc.vector.tensor_scalar(
                  out=pp, in0=y2, scalar1=C3 * ALPHA, scalar2=ALPHA,
                  op0=mybir.AluOpType.mult, op1=mybir.AluOpType.add,
              )
              tt = small.tile([P, A_PER_CHUNK], F32)
              nc.vector.tensor_mul(tt, y, pp)
              ng = small.tile([P, A_PER_CHUNK], F32)
              nc.vector.tensor_scalar_mul(out=ng, in0=tt, scalar1=-1.0)
              aa = small.tile([P, A_PER_CHUNK], F32)
              # aa = max(t, -t) + 1 = |t| + 1
              nc.vector.tensor_tensor(
                  out=aa, in0=tt, in1=ng, op=mybir.AluOpType.max,
              )
              nc.vector.tensor_scalar_add(out=aa, in0=aa, scalar1=1.0)
              rr = small.tile([P, A_PER_CHUNK], F32)
              nc.vector.reciprocal(rr, aa)
              qq = small.tile([P, A_PER_CHUNK], F32)
              nc.vector.tensor_mul(qq, tt, rr)
              hh = small.tile([P, A_PER_CHUNK], F32)
              nc.vector.tensor_scalar(
                  out=hh, in0=qq, scalar1=0.5, scalar2=0.5,
                  op0=mybir.AluOpType.mult, op1=mybir.AluOpType.add,
              )
              gg = small.tile([P, A_PER_CHUNK], F32)
              nc.vector.tensor_mul(gg, y, hh)
              # out = x + g (per-partition broadcast add, DVE tensor_scalar
              # runs fp16 single-src at 4x)
              for a in range(A_PER_CHUNK):
                  nc.vector.tensor_scalar_add(
                      out=ot[:, a, :], in0=xt[:, a, :],
                      scalar1=gg[:, a:a + 1],
                  )
              # out-stores ride the ACT HWDGE ring (qActDynamicHW)
              r0 = i * ROWS_PER_CHUNK
              nc.scalar.dma_start(
                  out=out[r0:r0 + ROWS_PER_CHUNK, :].rearrange(
                      "(p a) k -> p a k", a=A_PER_CHUNK),
                  in_=ot,
              )

          for i in range(NCHUNK + LOOKAHEAD):
              if i < NCHUNK:
                  head(i)
              if i >= LOOKAHEAD:
                  tail(i - LOOKAHEAD)

      # rep-level software pipeline: rep r+1's preamble (weight colsum,
      # AllReduce, v_b broadcast, bias) is emitted BEFORE rep r's main pass,
      # so its latency hides under rep r's streaming work.
      for _rep in range(reps):
          preamble(_rep)
          if _rep >= 1:
              main_pass(_rep - 1)
      main_pass(reps - 1)

    nc.compile()
    return nc


def get_nc(reps=1, sim=False, coll="dram", vb_ring="gpsimd"):
    key = ("nc", reps, sim, coll, vb_ring)
    if key not in _cached:
        _cached[key] = _build_bass(reps, sim=sim, coll=coll, vb_ring=vb_ring)
    return _cached[key]


def build_in_maps(inputs):
    x = np.asarray(inputs["x"], dtype=np.float32).astype(np.float16)
    import ml_dtypes
    weight = (np.asarray(inputs["weight"], dtype=np.float32) * WSCALE).astype(
        ml_dtypes.float8_e4m3)
    subtract = np.ascontiguousarray(
        np.asarray(inputs["subtract"], dtype=np.float32).reshape(1, N)
    ).astype(np.float16)
    in_maps = []
    for i in range(NCORES):
        in_maps.append({
            "x": np.ascontiguousarray(x[i * M_LOC:(i + 1) * M_LOC]),
            "w": np.ascontiguousarray(weight[i * W_LOC:(i + 1) * W_LOC]),
            "sub": subtract,
        })
    return in_maps


def run(inputs, trace=False):
    """Shard full inputs, run the SPMD kernel on 8 cores, gather full output.

    Returns (out, BassKernelResults)."""
    from concourse.bass_utils import run_bass_kernel_spmd

    in_maps = build_in_maps(inputs)
    nc = get_nc()
    res = run_bass_kernel_spmd(nc, in_maps, core_ids=list(range(NCORES)), trace=trace)
    out = np.concatenate([res.results[i]["out"] for i in range(NCORES)], axis=0)
    return out.astype(np.float32), res


def kernel(**inputs):
    out, _ = run(inputs, trace=False)
    return out
